# revision 1
# baseline (speedup 1.0000x reference)
"""TRN2 Bass kernel for nn_DecoderLayer_47175920779446.

Full decoder layer: qkv (mul-bias) -> 16-head attention -> +res -> LN ->
FFN(relu, mul-bias) -> +res -> LN, on x[2, 2048, 1024] fp32.

Sharding (8 cores): attention is sharded by (batch, 4 heads): core c handles
batch c//4, heads 4*(c%4)..4*(c%4)+3 over all 2048 tokens of its batch.
An 8-core AllToAll reshards attention output to token sharding (512 tokens
per core), under which LN1/FFN/LN2 run with fully replicated weights.

Precision: scores need ~fp32 accuracy (std ~256 feeding exp): q,k chain runs
fp32r (11-bit mantissa) projections, then an exact bf16 hi/lo split with a
2-matmul scheme: S = qh*kh + m_hat (main, K=65 with a fused bias row) plus
[qh;ql]*[kl;kh] (cross, K=128). V/P/FFN run bf16; residuals/LN run fp32.
"""
import contextlib
import numpy as np
import ml_dtypes

import concourse.bass as bass
import concourse.tile as tile
from concourse import bacc, mybir
from concourse.bass_utils import run_bass_kernel_spmd
from concourse.bass_interp import get_hw_module
from concourse.masks import make_identity

H, NH, HD, FF = 1024, 16, 64, 4096
B, T = 2, 2048
EPS = 1e-6
NCORES = 8
HPC = NH // 4          # 4 heads per core
TOK = (B * T) // NCORES  # 512 tokens per core
NKC = T // 128         # 16 key chunks
NG = T // 512          # 4 query groups
KCH = H // 128         # 8 contraction chunks for qkv
f32, f32r, bf16 = mybir.dt.float32, mybir.dt.float32r, mybir.dt.bfloat16
AF = mybir.ActivationFunctionType
ALU = mybir.AluOpType


def _round_mant(x, bits=11):
    xi = np.ascontiguousarray(x, np.float32).view(np.int32)
    shift = 23 - bits
    bias = (1 << (shift - 1)) - 1 + ((xi >> shift) & 1)
    xi = (xi + bias) & ~((1 << shift) - 1)
    return xi.view(np.float32)


def _build_program(sim_single=False):
    nc = bacc.Bacc("TRN2", target_bir_lowering=False, debug=False,
                   num_devices=1 if sim_single else NCORES)
    ap = {}
    ap["xT"] = nc.dram_tensor("xT", [H, T], f32r, kind="ExternalInput").ap()
    ap["xres"] = nc.dram_tensor("xres", [TOK, H], f32, kind="ExternalInput").ap()
    for w in ("wq", "wk", "wv"):
        ap[w] = nc.dram_tensor(w, [H, 4 * HD], f32r, kind="ExternalInput").ap()
    ap["w1"] = nc.dram_tensor("w1", [H, FF], bf16, kind="ExternalInput").ap()
    ap["w2"] = nc.dram_tensor("w2", [FF, H], bf16, kind="ExternalInput").ap()
    ap["lnw"] = nc.dram_tensor("lnw", [4, H], f32, kind="ExternalInput").ap()
    ap["bsel"] = nc.dram_tensor("bsel", [2], f32, kind="ExternalInput").ap()
    out_ap = nc.dram_tensor("out", [TOK, H], f32, kind="ExternalOutput").ap()

    with tile.TileContext(nc) as tc:
        ctx = contextlib.ExitStack()
        with ctx:
            const = ctx.enter_context(tc.tile_pool(name="const", bufs=1))
            dram = ctx.enter_context(tc.tile_pool(name="dram", bufs=1, space="DRAM"))

            ident = const.tile([128, 128], f32)
            make_identity(nc, ident[:])
            bs = const.tile([128, 2], f32)
            nc.sync.dma_start(bs[:], ap["bsel"].partition_broadcast(128))

            w1p = ctx.enter_context(tc.tile_pool(name="w1p", bufs=2))
            a2a_in = dram.tile([NCORES, TOK, 4 * HD], f32)
            a2a_out = dram.tile([NCORES, TOK, 4 * HD], f32)

            # ---------------- attention scope ----------------
            actx = contextlib.ExitStack()
            with actx:
                wpool = actx.enter_context(tc.tile_pool(name="wpool", bufs=1))
                qk = actx.enter_context(tc.tile_pool(name="qk", bufs=1))
                xgp = actx.enter_context(tc.tile_pool(name="xgp", bufs=4))
                sb = actx.enter_context(tc.tile_pool(name="sb", bufs=3))
                small = actx.enter_context(tc.tile_pool(name="small", bufs=4))
                psn = actx.enter_context(
                    tc.tile_pool(name="psn", bufs=2, space="PSUM"))
                pss = actx.enter_context(
                    tc.tile_pool(name="pss", bufs=2, space="PSUM"))
                pso = actx.enter_context(
                    tc.tile_pool(name="pso", bufs=1, space="PSUM"))
                psm = actx.enter_context(
                    tc.tile_pool(name="psm", bufs=1, space="PSUM"))

                w_sb = {}
                for w in ("wq", "wk", "wv"):
                    w_sb[w] = wpool.tile([128, KCH, 4 * HD], f32r, name=f"sb_{w}")
                    nc.sync.dma_start(
                        w_sb[w][:], ap[w].rearrange("(a p) c -> p a c", p=128))

                # per-head score operands
                til_q, til_k, cr_q, cr_k = {}, {}, {}, {}
                for h in range(HPC):
                    til_q[h] = qk.tile([65, T], bf16, name=f"til_q{h}", tag="tq", bufs=HPC)
                    til_k[h] = qk.tile([65, T], bf16, name=f"til_k{h}", tag="tk", bufs=HPC)
                    cr_q[h] = qk.tile([128, T], bf16, name=f"cr_q{h}", tag="cq", bufs=HPC)
                    cr_k[h] = qk.tile([128, T], bf16, name=f"cr_k{h}", tag="ck", bufs=HPC)
                    nc.gpsimd.memset(til_k[h][64:65, :], 1.0)
                vn = []
                for kc in range(NKC):
                    v = qk.tile([128, HPC, 65], bf16, name=f"vn{kc}", tag="vn", bufs=NKC)
                    nc.gpsimd.memset(v[:, :, 64:65], 1.0)
                    vn.append(v)

                # ---- QKV projection: all xg resident; K for all groups
                # first so attention's stage_a can begin ~18us earlier ----
                xgs = []
                for g in range(NG):
                    gsl = slice(512 * g, 512 * (g + 1))
                    xg = xgp.tile([128, KCH, 512], f32r, name=f"xg{g}", tag="xg", bufs=4)
                    nc.sync.dma_start(
                        xg[:], ap["xT"].rearrange("(a p) t -> p a t", p=128)[:, :, gsl])
                    xgs.append(xg)

                def proj_pass(name, til, cr, g):
                    gsl = slice(512 * g, 512 * (g + 1))
                    for hp in range(2):  # head pairs
                        p = pss.tile([128, 512], f32, tag="st", name="pqk")
                        for a in range(KCH):
                            nc.tensor.matmul(
                                p[:], w_sb[name][:, a, 128 * hp:128 * (hp + 1)],
                                xgs[g][:, a, :], start=(a == 0), stop=(a == KCH - 1))
                        for hl in range(2):
                            h = 2 * hp + hl
                            rows = slice(64 * hl, 64 * (hl + 1))
                            nc.scalar.activation(til[h][0:64, gsl], p[rows, :], AF.Copy)
                            if name == "wq":
                                hi_rows, lo_rows = slice(0, 64), slice(64, 128)
                            else:
                                hi_rows, lo_rows = slice(64, 128), slice(0, 64)
                            nc.sync.dma_start(cr[h][hi_rows, gsl], til[h][0:64, gsl])
                            nc.vector.scalar_tensor_tensor(
                                out=cr[h][lo_rows, gsl], in0=p[rows, :], scalar=1.0,
                                in1=til[h][0:64, gsl], op0=ALU.mult, op1=ALU.subtract)

                for g in range(NG):
                    proj_pass("wk", til_k, cr_k, g)
                for g in range(NG):
                    proj_pass("wq", til_q, cr_q, g)
                    for tt in range(4):  # V natural per token tile
                        kc = 4 * g + tt
                        p = pss.tile([128, 4 * HD], f32, tag="st", name="pv")
                        for a in range(KCH):
                            nc.tensor.matmul(
                                p[:], xgs[g][:, a, 128 * tt:128 * (tt + 1)],
                                w_sb["wv"][:, a, :], start=(a == 0), stop=(a == KCH - 1))
                        nc.scalar.activation(
                            vn[kc][:, :, 0:64],
                            p[:].rearrange("p (h d) -> p h d", h=HPC), AF.Copy)

                # ---- attention, software-pipelined over (head, group) units ----
                # stage A(unit): natural-S -> row max -> m_hat_neg row (PE+DVE)
                # stage B(unit): S~^T -> exp -> PV -> O out (PE+ACT+DVE)
                # emit A(i+2) between B(i-1) and B(i) so the m_hat chain is
                # hidden under two unit periods of PE work.
                units = [(h, g) for h in range(HPC) for g in range(NG)]

                def stage_a1(h, g):
                    # natural-S matmuls + DVE max reduces (no PE dependency on DVE)
                    mstage = small.tile([128, 4], f32, tag="mstage", name="mstage", bufs=2)
                    for qt in range(4):
                        qsl = slice(512 * g + 128 * qt, 512 * g + 128 * (qt + 1))
                        negmax = []
                        for half in range(2):
                            sn = psn.tile([128, 1024], f32, name="sn")
                            for j in range(2):
                                ks = slice(1024 * half + 512 * j,
                                           1024 * half + 512 * (j + 1))
                                nc.tensor.matmul(
                                    sn[:, 512 * j:512 * (j + 1)],
                                    til_q[h][0:64, qsl], til_k[h][0:64, ks],
                                    start=True, stop=True)
                            nm = small.tile([128, 1], f32, tag="nm", name="nm")
                            nc.vector.tensor_reduce(
                                nm[:], sn[:], axis=mybir.AxisListType.X,
                                op=ALU.max, negate=True)
                            negmax.append(nm)
                        nc.vector.tensor_tensor(
                            mstage[:, qt:qt + 1], negmax[0][:], negmax[1][:], ALU.min)
                    return mstage

                def stage_a2(h, g, mstage):
                    # emitted a period later so the PE transpose never waits on DVE
                    for qt in range(4):
                        qsl = slice(512 * g + 128 * qt, 512 * g + 128 * (qt + 1))
                        mt = psm.tile([1, 128], f32, tag="mt", name="mt")
                        nc.tensor.transpose(mt[:], mstage[:, qt:qt + 1], ident[:])
                        nc.vector.tensor_copy(til_q[h][64:65, qsl], mt[:])

                def stage_b(h, g):
                    gsl = slice(512 * g, 512 * (g + 1))
                    o_acc = pso.tile([65, 512], f32, name="o_acc")
                    pts = {}
                    PVLAG = 2

                    def pv(kc):
                        nc.tensor.matmul(o_acc[:], vn[kc][:, h, :], pts.pop(kc)[:],
                                         start=(kc == 0), stop=(kc == NKC - 1))

                    for kc in range(NKC):
                        ksl = slice(128 * kc, 128 * (kc + 1))
                        st = pss.tile([128, 512], f32, tag="st", name="st")
                        nc.tensor.matmul(st[:], til_k[h][0:65, ksl],
                                         til_q[h][0:65, gsl], start=True, stop=False)
                        nc.tensor.matmul(st[:], cr_k[h][:, ksl],
                                         cr_q[h][:, gsl], start=False, stop=True)
                        pt = sb.tile([128, 512], bf16, tag="pt", name="pt", bufs=6)
                        nc.scalar.activation(pt[:], st[:], AF.Exp)
                        pts[kc] = pt
                        if kc >= PVLAG:
                            pv(kc - PVLAG)
                    for kc in range(NKC - PVLAG, NKC):
                        pv(kc)
                    ot = sb.tile([65, 512], f32, tag="ot", name="ot")
                    nc.scalar.activation(ot[:], o_acc[:], AF.Copy)
                    # transpose to natural, scale by 1/denom, ship to a2a_in
                    for tt in range(4):
                        qt = 4 * g + tt
                        op_ = psm.tile([128, 65], f32, tag="mt", name="opt")
                        nc.tensor.transpose(
                            op_[:], ot[0:65, 128 * tt:128 * (tt + 1)],
                            ident[0:65, 0:65])
                        rc = small.tile([128, 1], f32, tag="rc", name="rc")
                        nc.vector.reciprocal(rc[:], op_[:, 64:65])
                        ob = sb.tile([128, HD], f32, tag="ob", name="ob", bufs=4)
                        nc.vector.tensor_scalar_mul(ob[:], op_[:, 0:64], rc[:])
                        j, r = qt // 4, qt % 4
                        for blk in (j, j + 4):
                            nc.sync.dma_start(
                                a2a_in[blk, 128 * r:128 * (r + 1),
                                       64 * h:64 * (h + 1)], ob[:])

                LOOKAHEAD = 3
                mstages, done_a2 = {}, set()
                for k in range(min(LOOKAHEAD, len(units))):
                    mstages[k] = stage_a1(*units[k])
                    stage_a2(*units[k], mstages.pop(k))
                    done_a2.add(k)
                for i, (h, g) in enumerate(units):
                    j = i + LOOKAHEAD
                    if j < len(units):
                        mstages[j] = stage_a1(*units[j])
                    j2 = i + LOOKAHEAD - 1
                    if j2 < len(units) and j2 not in done_a2:
                        stage_a2(*units[j2], mstages.pop(j2))
                        done_a2.add(j2)
                    stage_b(h, g)

            if sim_single:
                # timing stand-in for the 4MB AllToAll
                nc.sync.dma_start(a2a_out[:], a2a_in[:])
            else:
                nc.gpsimd.collective_compute(
                    "AllToAll", ALU.bypass,
                    replica_groups=[list(range(NCORES))],
                    ins=[a2a_in.opt()], outs=[a2a_out.opt()])

            # ---------------- FFN / LN scope ----------------
            fctx = contextlib.ExitStack()
            with fctx:
                fsb = fctx.enter_context(tc.tile_pool(name="fsb", bufs=2))
                o1p = fctx.enter_context(tc.tile_pool(name="o1p", bufs=1))
                w2p = fctx.enter_context(tc.tile_pool(name="w2p", bufs=1))
                fsm = fctx.enter_context(tc.tile_pool(name="fsm", bufs=4))
                psf = fctx.enter_context(
                    tc.tile_pool(name="psf", bufs=2, space="PSUM"))
                psg = fctx.enter_context(
                    tc.tile_pool(name="psg", bufs=2, space="PSUM"))

                lnbc = {}
                for i, nm in enumerate(("g1", "b1", "g2", "b2")):
                    lnbc[nm] = o1p.tile([128, H], f32, name=f"ln_{nm}", tag="lnbc", bufs=4)
                    nc.sync.dma_start(
                        lnbc[nm][:], ap["lnw"][i, :].partition_broadcast(128))

                out1 = o1p.tile([128, 4, H], f32, name="out1")       # natural, fp32
                out1T = o1p.tile([128, KCH, 512], bf16, name="out1T")  # transposed
                ht = o1p.tile([128, FF // 128, 512], bf16, name="ht")

                def layer_norm_to(dst, src, g_bc, b_bc, work):
                    """dst = gamma*(src-mean)/(std_unbiased+EPS)+beta, [128,H] tiles."""
                    stats = fsm.tile([128, 2, 6], f32, tag="stats", name="stats")
                    for hf in range(2):
                        nc.vector.bn_stats(stats[:, hf, :],
                                           src[:, 512 * hf:512 * (hf + 1)])
                    mv = fsm.tile([128, 2], f32, tag="mv", name="mv")
                    nc.vector.bn_aggr(mv[:], stats[:])
                    sd = fsm.tile([128, 1], f32, tag="sd", name="sd")
                    nc.scalar.activation(sd[:], mv[:, 1:2], AF.Sqrt,
                                         scale=float(H) / (H - 1))
                    nc.vector.tensor_scalar_add(sd[:], sd[:], EPS)
                    rs = fsm.tile([128, 1], f32, tag="rs", name="rs")
                    nc.vector.reciprocal(rs[:], sd[:])
                    nc.vector.tensor_scalar(out=work[:], in0=src[:],
                                            scalar1=mv[:, 0:1], scalar2=rs[:],
                                            op0=ALU.subtract, op1=ALU.mult)
                    nc.vector.tensor_mul(work[:], work[:], g_bc[:])
                    nc.vector.tensor_add(dst[:], work[:], b_bc[:])

                # LN1 over x + attn, per token tile
                for tt in range(4):
                    tsl = slice(128 * tt, 128 * (tt + 1))
                    at = fsb.tile([128, H], f32, tag="ta", name="at")
                    bt = fsb.tile([128, H], f32, tag="tb", name="bt")
                    for sl in range(4):
                        csl = slice(256 * sl, 256 * (sl + 1))
                        nc.sync.dma_start(at[:, csl], a2a_out[sl, tsl, :])
                        nc.sync.dma_start(bt[:, csl], a2a_out[sl + 4, tsl, :])
                    xt = fsb.tile([128, H], f32, tag="tc", name="xt")
                    nc.sync.dma_start(xt[:], ap["xres"][tsl, :])
                    nc.vector.tensor_scalar_mul(at[:], at[:], bs[:, 0:1])
                    nc.vector.scalar_tensor_tensor(
                        out=at[:], in0=bt[:], scalar=bs[:, 1:2], in1=at[:],
                        op0=ALU.mult, op1=ALU.add)
                    nc.vector.tensor_add(at[:], at[:], xt[:])
                    wk_ = fsb.tile([128, H], f32, tag="td", name="wk_")
                    layer_norm_to(out1[:, tt, :], at, lnbc["g1"], lnbc["b1"], wk_)
                    # transpose out1 tile -> out1T
                    for a in range(KCH):
                        tp = psg.tile([128, 128], f32, tag="tp", name="tp", bufs=2)
                        nc.tensor.transpose(
                            tp[:], out1[:, tt, 128 * a:128 * (a + 1)], ident[:])
                        nc.scalar.activation(
                            out1T[:, a, 128 * tt:128 * (tt + 1)], tp[:], AF.Copy)

                # FFN1: ht[f, t] = relu(W1^T x out1T), f-major
                for fb in range(KCH):  # 8 blocks of 512 ff cols
                    w1t = w1p.tile([128, KCH, 512], bf16, name="w1t")
                    nc.sync.dma_start(
                        w1t[:], ap["w1"].rearrange("(a p) f -> p a f", p=128)
                        [:, :, 512 * fb:512 * (fb + 1)])
                    for fq in range(4):  # 4 x 128 f-rows per block
                        ft = 4 * fb + fq
                        hp_ = psf.tile([128, 512], f32, tag="hp", name="hp", bufs=3)
                        for a in range(KCH):
                            nc.tensor.matmul(
                                hp_[:], w1t[:, a, 128 * fq:128 * (fq + 1)],
                                out1T[:, a, :], start=(a == 0), stop=(a == KCH - 1))
                        nc.scalar.activation(ht[:, ft, :], hp_[:], AF.Relu)

                # FFN2 token-major (both W2 halves resident) + fused LN2 tail
                f2 = o1p.tile([128, 4, H], f32, name="f2")
                w2ts = []
                for oc in range(2):
                    w2t = w2p.tile([128, FF // 128, 512], bf16, name=f"w2t{oc}",
                                   tag="w2t", bufs=2)
                    nc.sync.dma_start(
                        w2t[:], ap["w2"].rearrange("(a p) o -> p a o", p=128)
                        [:, :, 512 * oc:512 * (oc + 1)])
                    w2ts.append(w2t)
                for tt in range(4):
                    for oc in range(2):
                        acc = psf.tile([128, 512], f32, tag="o2", name="o2acc")
                        for ft in range(FF // 128):
                            nc.tensor.matmul(
                                acc[:], ht[:, ft, 128 * tt:128 * (tt + 1)],
                                w2ts[oc][:, ft, :], start=(ft == 0),
                                stop=(ft == FF // 128 - 1))
                        nc.scalar.activation(
                            f2[:, tt, 512 * oc:512 * (oc + 1)], acc[:], AF.Copy)
                    h2 = fsb.tile([128, H], f32, tag="ta", name="h2")
                    nc.vector.tensor_add(h2[:], out1[:, tt, :], f2[:, tt, :])
                    fin = fsb.tile([128, H], f32, tag="tb", name="fin")
                    wk2 = fsb.tile([128, H], f32, tag="tc", name="wk2")
                    layer_norm_to(fin, h2, lnbc["g2"], lnbc["b2"], wk2)
                    nc.sync.dma_start(out_ap[128 * tt:128 * (tt + 1), :], fin[:])

    nc.compile()
    if not sim_single:
        nc.m = get_hw_module(nc.m)
    return nc


_NC_CACHE = {}


def _get_program():
    if "nc" not in _NC_CACHE:
        _NC_CACHE["nc"] = _build_program()
    return _NC_CACHE["nc"]


def _prep_inputs(x, Wqkv, bqkv, W1, b1, W2, b2, gamma1, beta1, gamma2, beta2):
    """Host-side slicing/folding into per-core in_maps."""
    x = np.asarray(x, np.float32)
    Wqkv = np.asarray(Wqkv, np.float32)
    bqkv = np.asarray(bqkv, np.float32)
    d = np.arange(HD)
    hh = np.arange(NH)
    # qkv reshape in reference: [B,T,HD,3,NH] -> col = d*48 + k*16 + h
    cols = d[:, None, None] * (3 * NH) + np.arange(3)[None, :, None] * NH \
        + hh[None, None, :]
    Wq = Wqkv[:, cols[:, 0, :]] * (bqkv[cols[:, 0, :]] / np.sqrt(H))[None]
    Wk = Wqkv[:, cols[:, 1, :]] * bqkv[cols[:, 1, :]][None]
    Wv = Wqkv[:, cols[:, 2, :]] * bqkv[cols[:, 2, :]][None]
    # -> [H, HD, NH]; per-core head-major layout [H, 4*HD] (head-local major)
    Wq = np.transpose(Wq, (0, 2, 1))  # [H, NH, HD]
    Wk = np.transpose(Wk, (0, 2, 1))
    Wv = np.transpose(Wv, (0, 2, 1))
    W1e = (np.asarray(W1, np.float32) * np.asarray(b1, np.float32)[None]) \
        .astype(ml_dtypes.bfloat16)
    W2e = (np.asarray(W2, np.float32) * np.asarray(b2, np.float32)[None]) \
        .astype(ml_dtypes.bfloat16)
    lnw = np.stack([gamma1, beta1, gamma2, beta2]).astype(np.float32)
    xT = [_round_mant(np.ascontiguousarray(x[b].T)) for b in range(B)]
    in_maps = []
    for c in range(NCORES):
        b, grp = c // 4, c % 4
        heads = slice(4 * grp, 4 * grp + 4)
        in_maps.append({
            "xT": xT[b],
            "xres": np.ascontiguousarray(x[b, 512 * grp:512 * (grp + 1), :]),
            "wq": _round_mant(Wq[:, heads, :].reshape(H, 4 * HD)),
            "wk": _round_mant(Wk[:, heads, :].reshape(H, 4 * HD)),
            "wv": _round_mant(Wv[:, heads, :].reshape(H, 4 * HD)),
            "w1": W1e, "w2": W2e, "lnw": lnw,
            "bsel": np.array([1.0, 0.0] if b == 0 else [0.0, 1.0], np.float32),
        })
    return in_maps


def kernel(x, Wqkv, bqkv, W1, b1, W2, b2, gamma1, beta1, gamma2, beta2,
           _trace=False):
    nc = _get_program()
    in_maps = _prep_inputs(x, Wqkv, bqkv, W1, b1, W2, b2,
                           gamma1, beta1, gamma2, beta2)
    res = run_bass_kernel_spmd(nc, in_maps, core_ids=list(range(NCORES)),
                               trace=_trace)
    out = np.stack([res.results[c]["out"] for c in range(NCORES)])
    out = out.reshape(B, T, H).astype(np.float32)
    if _trace:
        kernel.last_results = res
    return out



# revision 91
# speedup vs baseline: 1.1433x; 1.1433x over previous
"""TRN2 Bass kernel for nn_DecoderLayer_47175920779446.

Full decoder layer: qkv (mul-bias) -> 16-head attention -> +res -> LN ->
FFN(relu, mul-bias) -> +res -> LN, on x[2, 2048, 1024] fp32.

Sharding (8 cores): attention is sharded by (batch, 4 heads): core c handles
batch c//4, heads 4*(c%4)..4*(c%4)+3 over all 2048 tokens of its batch.
Two head-pair-chunked 8-core AllToAlls reshard attention output to token
sharding (512 tokens per core); LN1/FFN/LN2 run data-parallel with
replicated weights. The hidden dim is column-permuted (host-side) so each
AllToAll chunk lands contiguously; the host inverse-permutes the output.

Precision: scores need ~fp32 accuracy (std ~256 feeding exp): q,k run as
fp32r (11-bit mantissa) end-to-end; S~ = k_r^T q_r is a single fp32r matmul
(1 cyc/row at free>=256) with the -rowmax folded in via a 65th ones/m_hat
row. The row max comes from a natural-orientation fp32r pass; max-combines
run on gpsimd, reduces on DVE. V/P/FFN run bf16; residuals/LN fp32.
"""
import contextlib
import numpy as np
import ml_dtypes

import concourse.bass as bass
import concourse.tile as tile
from concourse import bacc, mybir
from concourse.bass_utils import run_bass_kernel_spmd
from concourse.bass_interp import get_hw_module
from concourse.masks import make_identity

H, NH, HD, FF = 1024, 16, 64, 4096
B, T = 2, 2048
EPS = 1e-6
NCORES = 8
HPC = NH // 4          # 4 heads per core
TOK = (B * T) // NCORES  # 512 tokens per core
NKC = T // 128         # 16 key chunks
NG = T // 512          # 4 query groups
KCH = H // 128         # 8 contraction chunks for qkv
f32, f32r, bf16 = mybir.dt.float32, mybir.dt.float32r, mybir.dt.bfloat16
fp16 = mybir.dt.float16
AF = mybir.ActivationFunctionType
ALU = mybir.AluOpType

# hidden-dim permutation: chunk A = local heads {0,1} of each source core,
# chunk B = local heads {2,3}; each a2a chunk is then contiguous.
_PERM = np.concatenate(
    [256 * sl + 64 * hl + np.arange(64)
     for part in (0, 1) for sl in range(4) for hl in (2 * part, 2 * part + 1)])


def _round_mant(x, bits=11):
    xi = np.ascontiguousarray(x, np.float32).view(np.int32)
    shift = 23 - bits
    bias = (1 << (shift - 1)) - 1 + ((xi >> shift) & 1)
    xi = (xi + bias) & ~((1 << shift) - 1)
    return xi.view(np.float32)


def _build_program(sim_single=False):
    nc = bacc.Bacc("TRN2", target_bir_lowering=False, debug=False,
                   num_devices=1 if sim_single else NCORES)
    ap = {}
    ap["xT"] = nc.dram_tensor("xT", [H, T], f32r, kind="ExternalInput").ap()
    ap["xres"] = nc.dram_tensor("xres", [TOK, H], f32, kind="ExternalInput").ap()
    for w in ("wq", "wk", "wv"):
        ap[w] = nc.dram_tensor(w, [H, 4 * HD], f32r, kind="ExternalInput").ap()
    ap["w1"] = nc.dram_tensor("w1", [H, FF], fp16, kind="ExternalInput").ap()
    ap["w2"] = nc.dram_tensor("w2", [FF, H], fp16, kind="ExternalInput").ap()
    ap["lnw"] = nc.dram_tensor("lnw", [4, H], f32, kind="ExternalInput").ap()
    ap["bsel"] = nc.dram_tensor("bsel", [2], f32, kind="ExternalInput").ap()
    ap["ones"] = nc.dram_tensor("ones", [T], f32r, kind="ExternalInput").ap()
    out_ap = nc.dram_tensor("out", [TOK, H], f32, kind="ExternalOutput").ap()

    with tile.TileContext(nc) as tc:
        ctx = contextlib.ExitStack()
        with ctx:
            const = ctx.enter_context(tc.tile_pool(name="const", bufs=1))
            dram = ctx.enter_context(tc.tile_pool(name="dram", bufs=1, space="DRAM"))
            w1p = ctx.enter_context(tc.tile_pool(name="w1p", bufs=2))
            lnp = ctx.enter_context(tc.tile_pool(name="lnp", bufs=1))
            mid = contextlib.ExitStack()
            qk = mid.enter_context(tc.tile_pool(name="qk", bufs=1))

            ident = const.tile([128, 128], f32)
            make_identity(nc, ident[:])
            # warmup operand memset FIRST on the Pool queue so the PE p-state
            # warmup matmuls can start immediately.
            warm = const.tile([128, 512], bf16, name="warm")
            nc.gpsimd.memset(warm[:], 0.0)
            bs = const.tile([128, 2], f32)
            nc.sync.dma_start(bs[:], ap["bsel"].partition_broadcast(128))

            # a2a buffers: [x(batch-half dst), blk, tok, 128 cols]
            a2a_in = [dram.tile([2, 4, TOK, 128], f32, name=f"a2ai{i}")
                      for i in range(2)]
            a2a_out = [dram.tile([2, 4, TOK, 128], f32, name=f"a2ao{i}")
                       for i in range(2)]

            # LN1 persistent state
            lnbc = {}
            for i, nm in enumerate(("g1", "b1", "g2", "b2")):
                lnbc[nm] = lnp.tile([128, H], f32, name=f"ln_{nm}")
                nc.gpsimd.dma_start(
                    lnbc[nm][:], ap["lnw"][i, :].partition_broadcast(128))
            ln1pre = lnp.tile([128, NG, H], f32, name="ln1pre")
            for tt in range(4):
                nc.scalar.dma_start(
                    ln1pre[:, tt, :], ap["xres"][128 * tt:128 * (tt + 1), :])
            stats = lnp.tile([128, NG, 2, 6], f32, name="stats")

            # W1 rotating prefetch. The first bufs-many loads have no waits so
            # they go on the SP queue up front; the rest are emitted in the
            # FFN scope on the scalar queue (they park there waiting for FFN1
            # to free buffers, where it's harmless).
            W1BUFS = 3

            def w1_load(fb, eng):
                w1t = w1p.tile([128, KCH, 512], fp16, name=f"w1t{fb}",
                               tag="w1t", bufs=W1BUFS)
                eng.dma_start(
                    w1t[:], ap["w1"].rearrange("(a p) f -> p a f", p=128)
                    [:, :, 512 * fb:512 * (fb + 1)])
                return w1t

            w1ts = []  # first W1BUFS loads emitted after the proj DMAs

            # per-head score operands (f32r), V tiles (bf16)
            til_q, til_k = {}, {}
            for h in range(HPC):
                til_q[h] = qk.tile([128, T], f32r, name=f"til_q{h}", tag="tq", bufs=HPC)
                til_k[h] = qk.tile([128, T], f32r, name=f"til_k{h}", tag="tk", bufs=HPC)
                nc.sync.dma_start(til_k[h][127:128, :],
                                  ap["ones"].rearrange("(a t) -> a t", a=1))
            vn = []
            for kc in range(NKC):
                v = qk.tile([128, HPC, 65], fp16, name=f"vn{kc}", tag="vn", bufs=NKC)
                nc.gpsimd.memset(v[:, :, 64:65], 1.0)
                vn.append(v)

            # ---------------- warmup + projection scope ----------------
            pctx = contextlib.ExitStack()
            with pctx:
                wpool = pctx.enter_context(tc.tile_pool(name="wpool", bufs=1))
                xgp = pctx.enter_context(tc.tile_pool(name="xgp", bufs=1))
                pp = pctx.enter_context(
                    tc.tile_pool(name="pp", bufs=4, space="PSUM"))

                # PE p-state warmup: ~6us of dummy matmuls while DMAs stream
                for i in range(14):
                    pw = pp.tile([128, 512], f32, tag="st", name="pw")
                    nc.tensor.matmul(pw[:], warm[:, 0:128], warm[:],
                                     start=True, stop=True)

                # proj loads interleaved so K(g0) can start earliest
                w_sb = {}
                xgs = []

                def load_w(w):
                    w_sb[w] = wpool.tile([128, KCH, 4 * HD], f32r, name=f"sb_{w}")
                    nc.sync.dma_start(
                        w_sb[w][:], ap[w].rearrange("(a p) c -> p a c", p=128))

                def load_xg(g):
                    gsl = slice(512 * g, 512 * (g + 1))
                    xg = xgp.tile([128, KCH, 512], f32r, name=f"xg{g}",
                                  tag="xg", bufs=2)
                    for ah in range(2):
                        nc.sync.dma_start(
                            xg[:, 4 * ah:4 * (ah + 1), :],
                            ap["xT"].rearrange("(a p) t -> p a t", p=128)
                            [:, 4 * ah:4 * (ah + 1), gsl])
                    xgs.append(xg)

                load_w("wk")
                load_xg(0)
                load_w("wq")
                load_xg(1)
                load_w("wv")
                load_xg(2)
                load_xg(3)
                for fb in range(W1BUFS):
                    w1ts.append(w1_load(fb, nc.sync))

                def proj_qk(name, til, g):
                    gsl = slice(512 * g, 512 * (g + 1))
                    for hp_ in range(2):  # head pairs
                        p = pp.tile([128, 512], f32, tag="st", name="pqk")
                        for a in range(KCH):
                            nc.tensor.matmul(
                                p[:], w_sb[name][:, a, 128 * hp_:128 * (hp_ + 1)],
                                xgs[g][:, a, :], start=(a == 0), stop=(a == KCH - 1))
                        for hl in range(2):
                            h = 2 * hp_ + hl
                            nc.scalar.activation(
                                til[h][0:64, gsl], p[64 * hl:64 * (hl + 1), :],
                                AF.Copy)
                            if name == "wk":
                                # spare contraction rows carry the k-side
                                # fp32r rounding residual (63 of 64 dims)
                                nc.vector.tensor_tensor(
                                    til[h][64:127, gsl],
                                    p[64 * hl:64 * hl + 63, :],
                                    til[h][0:63, gsl], ALU.subtract)
                            else:
                                nc.scalar.activation(
                                    til[h][64:127, gsl],
                                    p[64 * hl:64 * hl + 63, :], AF.Copy)

                def proj_v(g):
                    for tt in range(4):
                        kc = 4 * g + tt
                        p = pp.tile([128, 4 * HD], f32, tag="st", name="pv")
                        for a in range(KCH):
                            nc.tensor.matmul(
                                p[:], xgs[g][:, a, 128 * tt:128 * (tt + 1)],
                                w_sb["wv"][:, a, :], start=(a == 0),
                                stop=(a == KCH - 1))
                        nc.vector.tensor_copy(
                            vn[kc][:, :, 0:64],
                            p[:].rearrange("p (h d) -> p h d", h=HPC))

                for g in range(NG):
                    proj_qk("wk", til_k, g)
                    proj_qk("wq", til_q, g)
                    proj_v(g)

            # ---------------- attention scope ----------------
            actx = contextlib.ExitStack()
            with actx:
                gtp = actx.enter_context(tc.tile_pool(name="gtp", bufs=1))
                sb = actx.enter_context(tc.tile_pool(name="sb", bufs=3))
                small = actx.enter_context(tc.tile_pool(name="small", bufs=4))
                psn = actx.enter_context(
                    tc.tile_pool(name="psn", bufs=2, space="PSUM"))
                pss = actx.enter_context(
                    tc.tile_pool(name="pss", bufs=2, space="PSUM"))
                pso = actx.enter_context(
                    tc.tile_pool(name="pso", bufs=1, space="PSUM"))
                psm = actx.enter_context(
                    tc.tile_pool(name="psm", bufs=1, space="PSUM"))

                units = [(h, g) for h in range(HPC) for g in range(NG)]

                def stage_a1_qt(h, g, qt, mstage):
                    # one query-tile of the natural-S max pass (generator:
                    # yields after each matmul pair). HW limits: gpsimd can't
                    # touch PSUM; DVE ops read at most one PSUM operand. So 3
                    # tiles get direct DVE reduces and the 4th goes through
                    # ACT as a log-sum-exp bound (exp(S/16) + accum_out row
                    # sum; 16*ln(sum) >= max, tight for peaked scores).
                    qsl = slice(512 * g + 128 * qt, 512 * g + 128 * (qt + 1))
                    nms = []
                    for half in range(2):
                        for j in range(2):
                            ks = slice(1024 * half + 512 * j,
                                       1024 * half + 512 * (j + 1))
                            sn = psn.tile([128, 512], f32, name="sn")
                            nc.tensor.matmul(
                                sn[:], til_q[h][0:64, qsl],
                                til_k[h][0:64, ks], start=True, stop=True)
                            nm = small.tile([128, 1], f32, tag="nm",
                                            name="nm", bufs=8)
                            nc.vector.tensor_reduce(
                                nm[:], sn[:], axis=mybir.AxisListType.X,
                                op=ALU.max, negate=True)
                            nms.append(nm)
                        yield
                    nc.vector.tensor_tensor(nms[0][:], nms[0][:], nms[1][:],
                                            ALU.min)
                    nc.vector.tensor_tensor(nms[2][:], nms[2][:], nms[3][:],
                                            ALU.min)
                    nc.vector.tensor_tensor(
                        mstage[:, qt:qt + 1], nms[0][:], nms[2][:], ALU.min)

                def stage_a2(h, g, mstage):
                    # [128,4] -> [4,128] -> m_hat row of til_q (via sbuf + DMA)
                    gsl = slice(512 * g, 512 * (g + 1))
                    mt4 = psm.tile([4, 128], f32, tag="mt", name="mt4")
                    nc.tensor.transpose(mt4[:], mstage[:, 0:4], ident[:])
                    sm4 = small.tile([4, 128], f32r, tag="sm4", name="sm4",
                                     bufs=3)
                    nc.vector.tensor_copy(sm4[:], mt4[:])
                    nc.sync.dma_start(
                        til_q[h][127:128, gsl]
                        .rearrange("a (t c) -> a t c", t=4), sm4[:])

                def stage_b(h, g, a1_feed=None):
                    gsl = slice(512 * g, 512 * (g + 1))
                    o_acc = pso.tile([65, 512], f32, name="o_acc")
                    pts = {}
                    PVLAG = 6  # in kc units; exp runs per kc-pair

                    def pv(kc):
                        pt = pts.pop(kc) if kc % 2 == 0 else pts[kc]
                        nc.tensor.matmul(
                            o_acc[:], vn[kc][:, h, :],
                            pt[:, 0, :] if kc % 2 == 0 else pt[:, 1, :],
                            start=(kc == 0), stop=(kc == NKC - 1))

                    for kp in range(NKC // 2):
                        if a1_feed is not None:
                            # interleave one lookahead-unit a1 matmul pair so
                            # PE keeps b-work queued while max-reduce
                            # consumers drain the a1 PSUM tiles
                            next(a1_feed, None)
                        st = pss.tile([128, 2, 512], f32, tag="st", name="st")
                        for j in range(2):
                            nc.tensor.matmul(
                                st[:, j, :],
                                til_k[h][0:128, 256 * kp + 128 * j:
                                         256 * kp + 128 * (j + 1)],
                                til_q[h][0:128, gsl], start=True, stop=True)
                        pt = sb.tile([128, 2, 512], fp16, tag="pt", name="pt",
                                     bufs=5)
                        nc.scalar.activation(pt[:], st[:], AF.Exp)
                        pts[2 * kp + 1] = pt
                        pts[2 * kp] = pt
                        for j in range(2):
                            kc = 2 * kp + j
                            if kc >= PVLAG:
                                pv(kc - PVLAG)
                    for kc in range(NKC - PVLAG, NKC):
                        pv(kc)
                    ot = sb.tile([65, 512], f32, tag="ot", name="ot")
                    nc.scalar.activation(ot[:], o_acc[:], AF.Copy)
                    # transpose to natural, scale by 1/denom, stage, ship
                    obg = sb.tile([128, 4, HD], f32, tag="obg", name="obg",
                                  bufs=3)
                    for tt in range(4):
                        op_ = psm.tile([128, 65], f32, tag="mt", name="opt")
                        nc.tensor.transpose(
                            op_[:], ot[0:65, 128 * tt:128 * (tt + 1)],
                            ident[0:65, 0:65])
                        rc = small.tile([128, 1], f32, tag="rc", name="rc")
                        nc.vector.reciprocal(rc[:], op_[:, 64:65])
                        nc.vector.tensor_scalar_mul(obg[:, tt, :],
                                                    op_[:, 0:64], rc[:])
                    chunk, hh = (0, h) if h < 2 else (1, h - 2)
                    for x in range(2):
                        nc.sync.dma_start(
                            a2a_in[chunk][x, g, :, 64 * hh:64 * (hh + 1)]
                            .rearrange("(t p) c -> p t c", p=128),
                            obg[:].rearrange("p t c -> p t c"))

                def emit_a2a_sim(chunk, g):
                    # per-(x,g) stand-in: unparks fast (waits only 2 units'
                    # ships) and holds the exclusive DMA resource briefly
                    for x in range(2):
                        nc.scalar.dma_start(a2a_out[chunk][x, g],
                                            a2a_in[chunk][x, g])

                def emit_a2a(chunk):
                    if not sim_single:
                        nc.gpsimd.collective_compute(
                            "AllToAll", ALU.bypass,
                            replica_groups=[list(range(NCORES))],
                            ins=[a2a_in[chunk][:].rearrange(
                                "x b t c -> (x b) t c").opt()],
                            outs=[a2a_out[chunk][:].rearrange(
                                "x b t c -> (x b) t c").opt()])

                def ln1b_block(g):
                    # one 128-col block of chunk B for all 4 token tiles:
                    # gather both batch-halves in one DMA + bsel-combine
                    csl = slice(512 + 128 * g, 512 + 128 * (g + 1))
                    for tt in range(4):
                        tsl = slice(128 * tt, 128 * (tt + 1))
                        at = sb.tile([128, 2, 128], f32, tag="atb", name="atb",
                                     bufs=4)
                        nc.sync.dma_start(
                            at[:], a2a_out[1][:, g, tsl, :]
                            .rearrange("x t c -> t x c"))
                        pre = ln1pre[:, tt, csl]
                        eng = nc.vector
                        eng.scalar_tensor_tensor(
                            out=pre, in0=at[:, 0, :], scalar=bs[:, 0:1],
                            in1=pre, op0=ALU.mult, op1=ALU.add)
                        eng.scalar_tensor_tensor(
                            out=pre, in0=at[:, 1, :], scalar=bs[:, 1:2],
                            in1=pre, op0=ALU.mult, op1=ALU.add)

                def ln1_partial(chunk, tt, pool, eng=None):
                    # gather chunk cols, bsel-combine into ln1pre, bn_stats
                    eng = eng or nc.vector
                    tsl = slice(128 * tt, 128 * (tt + 1))
                    at = pool.tile([128, 4, 128], f32, tag="at", name="at", bufs=2)
                    bt = pool.tile([128, 4, 128], f32, tag="bt", name="bt", bufs=2)
                    nc.scalar.dma_start(
                        at[:], a2a_out[chunk][0, :, tsl, :]
                        .rearrange("s t c -> t s c"))
                    nc.scalar.dma_start(
                        bt[:], a2a_out[chunk][1, :, tsl, :]
                        .rearrange("s t c -> t s c"))
                    pre = ln1pre[:, tt, 512 * chunk:512 * (chunk + 1)]
                    eng.scalar_tensor_tensor(
                        out=pre, in0=at[:].rearrange("p s c -> p (s c)"),
                        scalar=bs[:, 0:1], in1=pre, op0=ALU.mult, op1=ALU.add)
                    eng.scalar_tensor_tensor(
                        out=pre, in0=bt[:].rearrange("p s c -> p (s c)"),
                        scalar=bs[:, 1:2], in1=pre, op0=ALU.mult, op1=ALU.add)
                    nc.vector.bn_stats(stats[:, tt, chunk, :], pre)

                LOOKAHEAD = 3

                def a1_gen(j):
                    # yields per emitted matmul pair; finishes with a2
                    h, g = units[j]
                    mstage = small.tile([128, 4], f32, tag="mstage",
                                        name="mstage", bufs=4)
                    for qt in range(4):
                        for _ in stage_a1_qt(h, g, qt, mstage):
                            yield
                    stage_a2(h, g, mstage)

                feeds = []
                for k in range(min(LOOKAHEAD, len(units))):
                    for _ in a1_gen(k):
                        pass
                for i, (h, g) in enumerate(units):
                    j = i + LOOKAHEAD
                    feed = a1_gen(j) if j < len(units) else None
                    stage_b(h, g, a1_feed=feed)
                    if feed is not None:
                        for _ in feed:  # drain whatever stage_b didn't pull
                            pass
                    if sim_single:
                        if 4 <= i <= 7:
                            emit_a2a_sim(0, i - 4)
                        if 12 <= i <= 15:
                            emit_a2a_sim(1, i - 12)
                    elif i == 7:
                        emit_a2a(0)
                    if 9 <= i <= 12:
                        ln1_partial(0, i - 9, sb)
                    if 12 <= i <= 15 and sim_single:
                        # chunk-1 gathers+combines per g block, under the
                        # attention tail; only stats/normalize remain after
                        ln1b_block(i - 12)
                emit_a2a(1)
                if not sim_single:
                    for g in range(NG):
                        ln1b_block(g)
            mid.close()  # free til/vn SBUF before FFN pools allocate

            # ---------------- FFN / LN scope ----------------
            fctx = contextlib.ExitStack()
            with fctx:
                fsb = fctx.enter_context(tc.tile_pool(name="fsb", bufs=2))
                o1p = fctx.enter_context(tc.tile_pool(name="o1p", bufs=1))
                w2p = fctx.enter_context(tc.tile_pool(name="w2p", bufs=1))
                fsm = fctx.enter_context(tc.tile_pool(name="fsm", bufs=4))
                psf = fctx.enter_context(
                    tc.tile_pool(name="psf", bufs=2, space="PSUM"))
                psg = fctx.enter_context(
                    tc.tile_pool(name="psg", bufs=2, space="PSUM"))

                out1 = o1p.tile([128, 4, H], f32, name="out1")
                out1T = o1p.tile([128, KCH, 512], fp16, name="out1T")
                ht = o1p.tile([128, FF // 128, 512], fp16, name="ht")
                h2 = ln1pre  # dead after LN1 finish; reuse for the residual sum

                def ln_finish(dst, src, g_bc, b_bc, st2, eng=None):
                    """dst = gamma*(src-mean)/(std_unbiased+EPS)+beta."""
                    eng = eng or nc.vector
                    mv = fsm.tile([128, 2], f32, tag="mv", name="mv")
                    nc.vector.bn_aggr(mv[:], st2)
                    sd = fsm.tile([128, 1], f32, tag="sd", name="sd")
                    nc.scalar.activation(sd[:], mv[:, 1:2], AF.Sqrt,
                                         scale=float(H) / (H - 1))
                    nc.vector.tensor_scalar_add(sd[:], sd[:], EPS)
                    rs = fsm.tile([128, 1], f32, tag="rs", name="rs")
                    nc.vector.reciprocal(rs[:], sd[:])
                    wk_ = fsb.tile([128, H], f32, tag="wk2", name="wk2_")
                    nc.vector.tensor_scalar(out=wk_[:], in0=src,
                                            scalar1=mv[:, 0:1], scalar2=rs[:],
                                            op0=ALU.subtract, op1=ALU.mult)
                    # gamma/beta off DVE: frees it for the next tile's stats
                    eng.tensor_tensor(wk_[:], wk_[:], g_bc[:], ALU.mult)
                    eng.tensor_tensor(dst, wk_[:], b_bc[:], ALU.add)

                # LN1 finish per token tile, half-row granularity with
                # alternating gamma/beta engines; transposes start per half
                for tt in range(4):
                    eng = nc.vector
                    nc.vector.bn_stats(stats[:, tt, 1, :],
                                       ln1pre[:, tt, 512:1024])
                    mv = fsm.tile([128, 2], f32, tag="mv", name="mv")
                    nc.vector.bn_aggr(mv[:], stats[:, tt, :, :])
                    sd = fsm.tile([128, 1], f32, tag="sd", name="sd")
                    nc.scalar.activation(sd[:], mv[:, 1:2], AF.Sqrt,
                                         scale=float(H) / (H - 1))
                    nc.vector.tensor_scalar_add(sd[:], sd[:], EPS)
                    rs = fsm.tile([128, 1], f32, tag="rs", name="rs")
                    nc.vector.reciprocal(rs[:], sd[:])
                    for half in range(2):
                        hsl = slice(512 * half, 512 * (half + 1))
                        wk_ = fsb.tile([128, 512], f32, tag="wk", name="wk_",
                                       bufs=3)
                        nc.vector.tensor_scalar(
                            out=wk_[:], in0=ln1pre[:, tt, hsl],
                            scalar1=mv[:, 0:1], scalar2=rs[:],
                            op0=ALU.subtract, op1=ALU.mult)
                        eng.tensor_tensor(wk_[:], wk_[:],
                                          lnbc["g1"][:, hsl], ALU.mult)
                        eng.tensor_tensor(out1[:, tt, hsl], wk_[:],
                                          lnbc["b1"][:, hsl], ALU.add)
                        for a in range(4 * half, 4 * half + 4):
                            tp = psg.tile([128, 128], f32, tag="tp", name="tp",
                                          bufs=2)
                            nc.tensor.transpose(
                                tp[:], out1[:, tt, 128 * a:128 * (a + 1)],
                                ident[:])
                            nc.scalar.activation(
                                out1T[:, a, 128 * tt:128 * (tt + 1)], tp[:],
                                AF.Copy)

                # FFN1: ht[f, t] = relu(W1^T x out1T), token-half granularity
                for fb in range(W1BUFS, KCH):
                    w1ts.append(w1_load(fb, nc.scalar))
                for fb in range(KCH):
                    for fq in range(4):
                        ft = 4 * fb + fq
                        for th in range(2):
                            tsl = slice(256 * th, 256 * (th + 1))
                            hp_ = psf.tile([128, 256], f32, tag="hp", name="hp",
                                           bufs=3)
                            for a in range(KCH):
                                nc.tensor.matmul(
                                    hp_[:], w1ts[fb][:, a, 128 * fq:128 * (fq + 1)],
                                    out1T[:, a, tsl], start=(a == 0),
                                    stop=(a == KCH - 1))
                            nc.scalar.activation(ht[:, ft, tsl], hp_[:], AF.Relu)

                # W2 eighths prefetch (scalar queue; transfers slot in behind
                # the w1t rotation during FFN1, ready by FFN2)
                w2ts = []
                for oq in range(8):
                    w2t = w2p.tile([128, FF // 128, 128], fp16, name=f"w2t{oq}",
                                   tag="w2t", bufs=3)
                    nc.scalar.dma_start(
                        w2t[:], ap["w2"].rearrange("(a p) o -> p a o", p=128)
                        [:, :, 128 * oq:128 * (oq + 1)])
                    w2ts.append(w2t)

                # FFN2 eighth-major + residual add into h2 + incremental LN2
                # stats so only the last eighth's LN2 work sits in the tail
                st2b = fsm.tile([128, 4, 8, 6], f32, tag="st2", name="st2")
                for oq in range(8):
                    osl = slice(128 * oq, 128 * (oq + 1))
                    for tt in range(4):
                        acc = psf.tile([128, 128], f32, tag="o2", name="o2acc")
                        for ft in range(FF // 128):
                            nc.tensor.matmul(
                                acc[:], ht[:, ft, 128 * tt:128 * (tt + 1)],
                                w2ts[oq][:, ft, :], start=(ft == 0),
                                stop=(ft == FF // 128 - 1))
                        nc.vector.tensor_tensor(
                            h2[:, tt, osl], acc[:], out1[:, tt, osl], ALU.add)
                        nc.vector.bn_stats(st2b[:, tt, oq, :], h2[:, tt, osl])

                # LN2 + output (alternate gamma/beta engines so the 4 tail
                # chains pipeline across Pool and DVE)
                for tt in range(4):
                    fin = fsb.tile([128, H], f32, tag="fin", name="fin")
                    ln_finish(fin[:], h2[:, tt, :], lnbc["g2"], lnbc["b2"],
                              st2b[:, tt, :, :],
                              eng=nc.vector)
                    nc.sync.dma_start(out_ap[128 * tt:128 * (tt + 1), :], fin[:])

    nc.compile()
    if not sim_single:
        nc.m = get_hw_module(nc.m)
    return nc


_NC_CACHE = {}


def _get_program():
    if "nc" not in _NC_CACHE:
        _NC_CACHE["nc"] = _build_program()
    return _NC_CACHE["nc"]


def _prep_inputs(x, Wqkv, bqkv, W1, b1, W2, b2, gamma1, beta1, gamma2, beta2):
    """Host-side slicing/folding into per-core in_maps."""
    x = np.asarray(x, np.float32)
    Wqkv = np.asarray(Wqkv, np.float32)
    bqkv = np.asarray(bqkv, np.float32)
    d = np.arange(HD)
    hh = np.arange(NH)
    # qkv reshape in reference: [B,T,HD,3,NH] -> col = d*48 + k*16 + h
    cols = d[:, None, None] * (3 * NH) + np.arange(3)[None, :, None] * NH \
        + hh[None, None, :]
    Wq = Wqkv[:, cols[:, 0, :]] * (bqkv[cols[:, 0, :]] / np.sqrt(H))[None]
    Wk = Wqkv[:, cols[:, 1, :]] * bqkv[cols[:, 1, :]][None]
    Wv = Wqkv[:, cols[:, 2, :]] * bqkv[cols[:, 2, :]][None]
    # -> [H, HD, NH]; per-core head-major layout [H, 4*HD] (head-local major)
    Wq = np.transpose(Wq, (0, 2, 1))  # [H, NH, HD]
    Wk = np.transpose(Wk, (0, 2, 1))
    Wv = np.transpose(Wv, (0, 2, 1))
    W1e = (np.asarray(W1, np.float32) * np.asarray(b1, np.float32)[None]) \
        [_PERM, :].astype(np.float16)
    W2e = (np.asarray(W2, np.float32) * np.asarray(b2, np.float32)[None]) \
        [:, _PERM].astype(np.float16)
    lnw = np.stack([gamma1, beta1, gamma2, beta2]).astype(np.float32)[:, _PERM]
    lnw = np.ascontiguousarray(lnw)
    xT = [_round_mant(np.ascontiguousarray(x[b].T)) for b in range(B)]
    in_maps = []
    for c in range(NCORES):
        b, grp = c // 4, c % 4
        heads = slice(4 * grp, 4 * grp + 4)
        in_maps.append({
            "xT": xT[b],
            "xres": np.ascontiguousarray(
                x[b, 512 * grp:512 * (grp + 1), :][:, _PERM]),
            "wq": _round_mant(Wq[:, heads, :].reshape(H, 4 * HD)),
            "wk": _round_mant(Wk[:, heads, :].reshape(H, 4 * HD)),
            "wv": _round_mant(Wv[:, heads, :].reshape(H, 4 * HD)),
            "w1": W1e, "w2": W2e, "lnw": lnw,
            "bsel": np.array([1.0, 0.0] if b == 0 else [0.0, 1.0], np.float32),
            "ones": np.ones(T, np.float32),
        })
    return in_maps


def kernel(x, Wqkv, bqkv, W1, b1, W2, b2, gamma1, beta1, gamma2, beta2,
           _trace=False):
    nc = _get_program()
    in_maps = _prep_inputs(x, Wqkv, bqkv, W1, b1, W2, b2,
                           gamma1, beta1, gamma2, beta2)
    res = run_bass_kernel_spmd(nc, in_maps, core_ids=list(range(NCORES)),
                               trace=_trace)
    out = np.stack([res.results[c]["out"] for c in range(NCORES)])
    out = out.reshape(B, T, H)
    full = np.empty_like(out)
    full[:, :, _PERM] = out
    if _trace:
        kernel.last_results = res
    return full.astype(np.float32)
